# revision 24
# baseline (speedup 1.0000x reference)
"""ParallelRetention Trainium2 Bass kernel (v3).

Problem (per [b,h] slice, B=2 H=16 S=2048 D=64):
    decay  = omask / sqrt(rowsum(omask))          (per-row rsqrt scale)
    ret    = (q @ k^T) * decay
    denom  = clip(|rowsum(ret)|, 1, inf)
    out    = (ret / denom) @ v
Restructured:
    augT   = [v | 1]^T @ (scores * omask)^T       # [65, S] per (b,h)
    msum   = rowsum(omask) = [0..0|1]^T @ omask^T  # extra PE stream
    out[q] = augT[0:64, q] / max(|augT[64, q]|, sqrt(msum[q]))

Structure (v6, ~275us; v5 was ~287us, original baseline 372us):
  - Scores computed transposed ([k, q]) with k-pairs row-packed so the two
    K=64 matmuls of a pair run in disjoint PE row groups. qT/kT are fp16
    (ACT casts on the PSUM->SBUF copy): the pair's two moving streams
    share the 4B/lane/cycle SBUF->PE bus, so 2-byte operands let the pair
    stream in ~316ns instead of fp32's 428ns. fp16 (not bf16: bf16 q/k
    fails the 2e-2 gate at 0.032; fp16 lands at 0.0043).
  - omaskT built by PE transposes + ACT PSUM->SBUF copies, software-
    pipelined into the previous chunk's matmul stream (build for ch+1
    interleaves with stream of ch); omask DMA prefetches a chunk ahead.
  - score*omask multiplies on DVE directly from PSUM (f32r out); the
    second (aug) matmul consumes them with [v|1] stationary; aug matmuls
    are deferred AUG_DEFER jj-slots so the PE never waits on the DVE.
  - msum comes from a second matmul stream over the omaskT tiles with an
    M=66 [0...0|1] stationary: cols 0-64 add zeros onto the aug rows and
    col 65 accumulates rowsum(omask) into partition 65 of b0's aug bank -
    no ACT/DVE row-sum pass at all. The postproc transpose carries msum
    out alongside augT (its matmuls run start=False onto an explicitly
    zeroed row, and begin only after both aug-group starts).
  - Each chunk's postprocess is deferred into the next chunk's stream
    (issued at jj=0, before that chunk's first deferred aug flush, which
    keeps aug-bank reuse ordering correct); qT/kT/va are triple-buffered
    so the next head's prep overlaps the current head's tail.
  - GPSIMD offload and DVE-accumulate msum were measured and are net
    losses (PSUM-copy tax / 1x CACHE_REDUCE); knobs remain for reference.

Sharding: 16 heads / 8 cores = 2 heads per core, both batches on the same
core (omask is per-head, halving omask traffic per core). SPMD: one NEFF,
per-core input slices.
"""

import os

import numpy as np

B = 2
H = 16
S = 2048
D = 64
N_CORES = 8
HC = H // N_CORES
P = 128
QT = S // P
KT = S // P
QC = 512
NCH = S // QC
TPC = QC // P

GP_QUADS = int(os.environ.get("KRN_GP_QUADS", "0"))   # 0..8 gpsimd quads/chunk
GP_DEFER = int(os.environ.get("KRN_GP_DEFER", "2"))   # jj slots to defer quad
AUG_DEFER = int(os.environ.get("KRN_AUG_DEFER", "3"))  # jj slots to defer the
                                                       # DVE-path aug MMs
SG8 = os.environ.get("KRN_SG8", "0") == "1"           # 2 k-blocks per stage
PIPE = os.environ.get("KRN_PIPE", "1") == "1"         # pipelined build
MSUM_MM = os.environ.get("KRN_MSUM_MM", "1") == "1"   # msum via PE stream
MSUM_DVE = int(os.environ.get("KRN_MSUM_DVE", "1"))   # msum tiles/chunk on
                                                      # DVE when MSUM_MM=0
N_WARMUP = int(os.environ.get("KRN_WARMUP", "12"))
FILLERS = int(os.environ.get("KRN_FILLERS", "0"))

_NC_CACHE = {}


def _build_nc():
    import concourse.mybir as mybir
    import concourse.tile as tile
    from concourse import bacc
    from concourse.masks import make_identity

    F32R = mybir.dt.float32r
    F32 = mybir.dt.float32
    BF16 = mybir.dt.bfloat16
    FP16 = mybir.dt.float16
    MULT = mybir.AluOpType.mult
    ADD = mybir.AluOpType.add

    KPG = 2 if SG8 else 1          # k-blocks per transpose stage group
    NGRP = KT // KPG               # stage groups per chunk (8 or 16)
    PGSZ = 8 if SG8 else 4         # transposes per prep stage group
    MCOL = 64                      # PE array col position of the msum
                                   # stationary; msum row = partition 65

    # build-step placement across the 8 jj slots of the previous chunk's
    # stream: group counts per jj (sums to NGRP)
    if SG8:
        GRP_AT = [0, 0, 2, 1, 1, 1, 1, 2]
    else:
        GRP_AT = [0, 0, 2, 3, 3, 3, 3, 2]
    MS_AT = [0, 0, 3, 3, 3, 3, 2, 2]     # msum matmuls per jj (sums to 16)
    MSUM_AT = {3: 0, 5: 1, 6: 2, 7: 3}   # jj -> onat tile (MSUM_MM=0 path)

    nc = bacc.Bacc("TRN2", target_bir_lowering=False, debug=False,
                   num_devices=N_CORES)

    q_d = nc.dram_tensor("q", [B, HC, S, D], F32, kind="ExternalInput")
    k_d = nc.dram_tensor("k", [B, HC, S, D], F32, kind="ExternalInput")
    v_d = nc.dram_tensor("v", [B, HC, S, D], F32, kind="ExternalInput")
    om_d = nc.dram_tensor("omask", [HC, S, S], F32, kind="ExternalInput")
    out_d = nc.dram_tensor("out", [B, HC, S, D], F32, kind="ExternalOutput")

    with tile.TileContext(nc) as tc:
        with (
            tc.tile_pool(name="const", bufs=1) as const_pool,
            tc.tile_pool(name="onat", bufs=4) as onat_pool,
            tc.tile_pool(name="omt", bufs=3) as omt_pool,
            tc.tile_pool(name="qkv", bufs=2) as qkv_pool,
            tc.tile_pool(name="work", bufs=min(4 + 2 * AUG_DEFER, 8)) as work_pool,
            tc.tile_pool(name="quad", bufs=2) as quad_pool,
            tc.tile_pool(name="small", bufs=4) as small_pool,
            tc.tile_pool(name="outp", bufs=2) as out_pool,
            tc.tile_pool(name="ps_sc", bufs=2, space="PSUM") as ps_sc,
            tc.tile_pool(name="ps_stage", bufs=(1 if SG8 else 2),
                         space="PSUM") as ps_stage,
            tc.tile_pool(name="ps_aug", bufs=2, space="PSUM") as ps_aug,
        ):
            ident_f = const_pool.tile([P, P], F32, tag="ident_f")
            make_identity(nc, ident_f)

            # [0 ... 0 | 1] stationary (M=66) for the msum matmul stream in
            # plain 128x128 mode: cols 0-64 add +0 onto the aug rows, col
            # 65 accumulates rowsum(omask) into partition 65. f32r so the
            # stream runs in single-pass full-rate fp32 mode.
            z66_f = const_pool.tile([P, D + 2], F32, tag="z66_f")
            nc.vector.memset(z66_f, 0.0)
            nc.vector.memset(z66_f[:, D + 1:D + 2], 1.0)
            z66 = const_pool.tile([P, D + 2], F32R, tag="z66")
            nc.vector.tensor_copy(z66, z66_f)
            tinyb = const_pool.tile([P, 1], F32, tag="tinyb")
            nc.vector.memset(tinyb, 1e-30)

            # dummy outputs for the MSUM_MM=0 accumulate path (never read)
            mdum_dve = None
            mdum_act = None
            if not MSUM_MM:
                F8 = mybir.dt.float8e4
                if MSUM_DVE > 0:
                    mdum_dve = const_pool.tile([P, S], F8, tag="mdum_dve")
                if MSUM_DVE < TPC:
                    mdum_act = const_pool.tile([P, S], F8, tag="mdum_act")

            # PE warmup: back-to-back matmuls so the HAM clock gate lifts
            # (1.2 -> 2.4 GHz) before the real stream; overlaps first DMAs.
            warm_w = const_pool.tile([P, P], F32R, tag="warm_w")
            nc.vector.tensor_copy(warm_w, ident_f)
            warm_xf = const_pool.tile([P, QC], F32, tag="warm_xf")
            nc.vector.memset(warm_xf, 1.0)
            warm_x = const_pool.tile([P, QC], F32R, tag="warm_x")
            nc.vector.tensor_copy(warm_x, warm_xf)
            if N_WARMUP:
                warm_ps = ps_aug.tile([P, QC], F32, tag="aug")
                for _ in range(N_WARMUP):
                    nc.tensor.matmul(warm_ps, warm_w, warm_x,
                                     start=True, stop=True)
                warm_sink = small_pool.tile([P, 1], F32, tag="warm_sink")
                nc.vector.tensor_copy(warm_sink, warm_ps[:, 0:1])

            def prep_inputs(b, h):
                # q tiles with the d-column block duplicated ([p, t, 128] =
                # [q | q]) and k tiles packed pairwise; a [128,128] PE
                # transpose of each yields qT duplicated into both partition
                # halves and kT pairs split 0-63/64-127 so the two K=64
                # score matmuls of a pair run in disjoint PE row-groups.
                qsrc = q_d[b, h].rearrange("(t p) d -> p t d", p=P)
                qn2 = qkv_pool.tile([P, QT, P], F32, tag="qn")
                nc.sync.dma_start(out=qn2[:, :, 0:D], in_=qsrc)
                nc.sync.dma_start(out=qn2[:, :, D:2 * D], in_=qsrc)
                kn2 = qkv_pool.tile([P, KT // 2, 2, D], F32, tag="kn")
                nc.sync.dma_start(
                    out=kn2,
                    in_=k_d[b, h].rearrange(
                        "(jj two p) d -> p jj two d", p=P, two=2))
                vn = qkv_pool.tile([P, KT, D], F32, tag="vn")
                nc.sync.dma_start(
                    out=vn,
                    in_=v_d[b, h].rearrange("(t p) d -> p t d", p=P))
                va = qkv_pool.tile([P, KT, D + 1], F32R, tag="va", bufs=3)
                nc.vector.tensor_copy(va[:, :, 0:D], vn)
                onesf = small_pool.tile([P, KT], F32, tag="onesf")
                nc.vector.memset(onesf, 1.0)
                nc.vector.tensor_copy(
                    va[:, :, D:D + 1].rearrange("p t o -> p (t o)"), onesf)

                # qT/kT in bf16: halves the score matmuls' moving-stream
                # bytes so the two row-group-packed K=64 matmuls of a pair
                # can stream concurrently
                qT = qkv_pool.tile([P, S], FP16, tag="qT", bufs=3)
                for g in range(QT // PGSZ):
                    stg = ps_stage.tile([P, PGSZ, P], F32, tag="stage")
                    for i in range(PGSZ):
                        nc.tensor.transpose(
                            stg[:, i, :], qn2[:, g * PGSZ + i, :], ident_f)
                    nc.scalar.copy(
                        out=qT[:, g * PGSZ * P:(g + 1) * PGSZ * P]
                            .rearrange("d (i c) -> d i c", c=P),
                        in_=stg)
                kT = qkv_pool.tile([P, KT // 2, P], FP16, tag="kT", bufs=3)
                kg = min(PGSZ, KT // 2)
                for g in range((KT // 2) // kg):
                    stg = ps_stage.tile([P, kg, P], F32, tag="stage")
                    for i in range(kg):
                        nc.tensor.transpose(
                            stg[:, i, :],
                            kn2[:, g * kg + i, :, :]
                                .rearrange("p two d -> p (two d)"),
                            ident_f)
                    nc.scalar.copy(
                        out=kT[:, g * kg:(g + 1) * kg, :], in_=stg)
                return qT, kT, va

            # GPSIMD quad assignment: (b, quad) pairs, spread across jj & b
            gps_set = set(
                [(0, 1), (1, 2), (1, 0), (0, 2), (0, 0), (1, 1), (0, 3),
                 (1, 3)][:GP_QUADS])

            # postproc closures deferred into the next chunk's stream so
            # their ACT/DVE/PE chains overlap instead of serializing at
            # chunk boundaries
            pending_post = []

            def flush_post():
                while pending_post:
                    pending_post.pop(0)()

            for h in range(HC):
                prepped = [prep_inputs(b, h) for b in range(B)]
                msum = None
                if not MSUM_MM:
                    msum = small_pool.tile([P, QT], F32, tag="msum")

                def msum_op(onats, ch, t):
                    qt = ch * TPC + t
                    if t < MSUM_DVE:
                        nc.vector.tensor_scalar(
                            mdum_dve, onats[t], 1.0, 0.0, MULT, ADD,
                            accum_out=msum[:, qt:qt + 1])
                    else:
                        nc.scalar.activation(
                            mdum_act, onats[t],
                            mybir.ActivationFunctionType.Copy,
                            accum_out=msum[:, qt:qt + 1])

                def issue_onat_dma(ch):
                    onats = []
                    for t in range(TPC):
                        onat = onat_pool.tile([P, S], F32, tag="onat")
                        r0 = ch * QC + t * P
                        nc.sync.dma_start(
                            out=onat, in_=om_d[h, r0:r0 + P, :])
                        onats.append(onat)
                    return onats

                def alloc_omts():
                    # t-major layout [P, t, j8, c]: each build step (one
                    # onat row-tile, 4 k-blocks) writes a CONTIGUOUS
                    # [P, 4, 128] slab, and a step depends on only ONE
                    # omask DMA tile, so the build starts as soon as the
                    # first tile lands instead of waiting for all four.
                    return [omt_pool.tile([P, TPC, KT // 2, P], F32R,
                                          tag="omt", name=f"omt{i}")
                            for i in range(2)]

                def make_group_steps(onats, omts):
                    """16 closures, one per (row-tile t, k-quad jb)."""
                    def make_step(t, jb):
                        def step():
                            j0 = jb * 4
                            stg = ps_stage.tile([P, 4, P], F32, tag="stage")
                            for i in range(4):
                                j = j0 + i
                                nc.tensor.transpose(
                                    stg[:, i, :],
                                    onats[t][:, j * P:(j + 1) * P],
                                    ident_f)
                            half = j0 // (KT // 2)
                            j8 = j0 % (KT // 2)
                            nc.scalar.copy(
                                out=omts[half][:, t, j8:j8 + 4, :],
                                in_=stg)
                        return step
                    return [make_step(t, jb)
                            for t in range(TPC) for jb in range(4)]

                def full_build(ch):
                    onats = issue_onat_dma(ch)
                    omts = alloc_omts()
                    for step in make_group_steps(onats, omts):
                        step()
                    if not MSUM_MM:
                        for t in range(TPC):
                            msum_op(onats, ch, t)
                    return omts

                # chunk 0 of each head: built as its own phase (overlaps
                # warmup / previous head's stream tail via Tile scheduling)
                omts = full_build(0)

                for ch in range(NCH):
                    gsteps, omts_n, onats_n = [], None, None
                    if ch + 1 < NCH:
                        onats_n = issue_onat_dma(ch + 1)
                        omts_n = alloc_omts()
                        gsteps = make_group_steps(onats_n, omts_n)
                        if not PIPE:
                            for step in gsteps:
                                step()
                            gsteps = []

                    def omt_j(j):
                        # [P, t, c] view of k-block j (strided over t)
                        half = j // (KT // 2)
                        j8 = j % (KT // 2)
                        return omts[half][:, :, j8, :]

                    def omt_at(jj, n=2):
                        # [P, j, t, c] view matching the score pair layout
                        half = (jj * 2) // (KT // 2)
                        j8 = (jj * 2) % (KT // 2)
                        return omts[half][:, :, j8:j8 + n, :].rearrange(
                            "p t j c -> p j t c")

                    aug_ps = {}
                    pend = {}
                    started = {}
                    issued = {b: 0 for b in range(B)}
                    deferred = []
                    for b in range(B):
                        aug_b = ps_aug.tile([P, QC], F32, tag="aug",
                                            name=f"aug{b}")
                        aug_ps[b] = aug_b
                        pend[b] = None
                        started[b] = False

                    def mm2(b, j, src_ap):
                        va = prepped[b][2]
                        issued[b] += 1
                        nc.tensor.matmul(
                            aug_ps[b][0:D + 1, :], va[:, j, :], src_ap,
                            start=not started[b],
                            stop=(issued[b] == KT),
                            skip_group_check=True)
                        started[b] = True

                    def flush_deferred(jj_now):
                        for item in list(deferred):
                            fjj, b, mms = item
                            if jj_now < fjj:
                                continue
                            for j, ap in mms:
                                mm2(b, j, ap)
                            deferred.remove(item)

                    gi = 0
                    ms_done = 0
                    for jj in range(KT // 2):
                        for b in range(B):
                            qT, kT, va = prepped[b]
                            sc = ps_sc.tile([P, 2, QC], F32, tag="scores")
                            for j2 in range(2):
                                base = j2 * D
                                nc.tensor.matmul(
                                    sc[:, j2, :], kT[base:base + D, jj, :],
                                    qT[base:base + D,
                                       ch * QC:(ch + 1) * QC],
                                    start=True, stop=True)
                            if (b, jj // 2) in gps_set:
                                if pend[b] is None:
                                    scq = quad_pool.tile([P, 4, QC], F32,
                                                         tag="scq")
                                    retq = quad_pool.tile([P, 4, QC], F32R,
                                                          tag="retq")
                                    pend[b] = (scq, retq, jj)
                                scq, retq, jj0 = pend[b]
                                off = (jj - jj0) * 2
                                nc.scalar.copy(
                                    out=scq[:, off:off + 2, :], in_=sc)
                                if off == 2:
                                    nc.gpsimd.tensor_mul(
                                        retq.rearrange(
                                            "p j (t c) -> p j t c", c=P),
                                        scq.rearrange(
                                            "p j (t c) -> p j t c", c=P),
                                        omt_at(jj0, 4))
                                    deferred.append((
                                        jj + GP_DEFER, b,
                                        [(jj0 * 2 + jq, retq[:, jq, :])
                                         for jq in range(4)]))
                                    pend[b] = None
                            else:
                                ret = work_pool.tile([P, 2, QC], F32R,
                                                     tag="ret")
                                nc.vector.tensor_mul(
                                    ret.rearrange("p j (t c) -> p j t c",
                                                  c=P),
                                    sc.rearrange("p j (t c) -> p j t c",
                                                 c=P),
                                    omt_at(jj))
                                mms = [(jj * 2 + j2, ret[:, j2, :])
                                       for j2 in range(2)]
                                if AUG_DEFER:
                                    deferred.append(
                                        (jj + AUG_DEFER, b, mms))
                                else:
                                    for j, ap in mms:
                                        mm2(b, j, ap)
                            flush_deferred(jj)
                        if jj == 0:
                            # previous chunk's postproc drops in here so it
                            # overlaps this stream. Must precede this
                            # chunk's first aug flush (jj>=1) so the aug
                            # PSUM buffer reuse ordering stays correct.
                            flush_post()
                            if MSUM_MM:
                                # zero the msum accumulator row (its
                                # matmuls all run start=False). Partition
                                # base must be 32-aligned: clear 64-65; row
                                # 64 is re-initialized by the aug group's
                                # start.
                                nc.vector.memset(
                                    aug_ps[0][MCOL:MCOL + 2, :], 0.0)
                        if MSUM_MM:
                            # msum stream: M=66 [0...0|1] stationary
                            # accumulating rowsum(omask) into partition 65
                            # of b0's aug bank (cols 0-64 add zeros). All
                            # start=False onto the explicitly-zeroed
                            # partition; begins at jj=2, after both
                            # aug-group starts have executed.
                            n_ms = MS_AT[jj]
                            for j in range(ms_done, ms_done + n_ms):
                                nc.tensor.matmul(
                                    aug_ps[0][0:D + 2, :], z66,
                                    omt_j(j),
                                    start=False, stop=(j == KT - 1),
                                    skip_group_check=True)
                            ms_done += n_ms
                        if PIPE and gsteps:
                            for _ in range(GRP_AT[jj]):
                                if gi < len(gsteps):
                                    gsteps[gi]()
                                    gi += 1
                        if (not MSUM_MM and onats_n is not None
                                and jj in MSUM_AT):
                            msum_op(onats_n, ch + 1, MSUM_AT[jj])
                        if FILLERS:
                            fill = ps_stage.tile([P, D], F32, tag="stage")
                            for _ in range(FILLERS):
                                nc.tensor.matmul(
                                    fill, warm_w, warm_x[:, 0:D],
                                    start=True, stop=True)
                    flush_deferred(10 ** 9)

                    def make_post(h, ch, aug_ps, msum):
                        def post():
                            msq = small_pool.tile([P, TPC], F32, tag="msq")
                            for b in range(B):
                                # postproc: [65, QC] (+ msum row) -> scaled
                                # [q, d] output
                                hi = (MCOL + 2 if (b == 0 and MSUM_MM)
                                      else D + 1)
                                augs = out_pool.tile([P, QC], F32,
                                                     tag="augs")
                                nc.scalar.copy(out=augs[0:hi, :],
                                               in_=aug_ps[b][0:hi, :])
                                autp = ps_stage.tile([P, TPC, MCOL + 2],
                                                     F32, tag="stage")
                                for t in range(TPC):
                                    nc.tensor.transpose(
                                        autp[:, t, 0:hi],
                                        augs[0:hi, t * P:(t + 1) * P],
                                        ident_f[0:hi, 0:hi])
                                if b == 0 and MSUM_MM:
                                    # msq = sqrt(msum + tiny)
                                    nc.scalar.activation(
                                        msq,
                                        autp[:, :, MCOL + 1:MCOL + 2]
                                        .rearrange("p t o -> p (t o)"),
                                        mybir.ActivationFunctionType.Sqrt,
                                        bias=tinyb[:, 0:1])
                                elif b == 0:
                                    nc.scalar.activation(
                                        msq,
                                        msum[:, ch * TPC:(ch + 1) * TPC],
                                        mybir.ActivationFunctionType.Sqrt,
                                        bias=tinyb[:, 0:1])
                                scal = small_pool.tile([P, TPC], F32,
                                                       tag="scal",
                                                       name=f"scal{b}")
                                nc.scalar.activation(
                                    scal,
                                    autp[:, :, D:D + 1].rearrange(
                                        "p t o -> p (t o)"),
                                    mybir.ActivationFunctionType.Abs)
                                nc.vector.tensor_max(scal, scal, msq)
                                nc.vector.reciprocal(scal, scal)
                                ob = out_pool.tile([P, TPC, D], F32,
                                                   tag="ob")
                                for t in range(TPC):
                                    # per-partition scale: native on ACT
                                    nc.scalar.activation(
                                        ob[:, t, :], autp[:, t, 0:D],
                                        mybir.ActivationFunctionType.Copy,
                                        scale=scal[:, t:t + 1])
                                nc.sync.dma_start(
                                    out=out_d[b, h,
                                              ch * QC:(ch + 1) * QC, :]
                                    .rearrange("(t p) d -> p t d", p=P),
                                    in_=ob)
                        return post

                    pending_post.append(make_post(h, ch, aug_ps, msum))

                    omts = omts_n
            flush_post()

    nc.compile()
    return nc


def _get_nc():
    if "nc" not in _NC_CACHE:
        _NC_CACHE["nc"] = _build_nc()
    return _NC_CACHE["nc"]


def kernel(q, k, v, omask, _trace=False):
    from concourse.bass_utils import run_bass_kernel_spmd

    nc = _get_nc()
    in_maps = []
    for c in range(N_CORES):
        hs = slice(c * HC, (c + 1) * HC)
        in_maps.append({
            "q": np.ascontiguousarray(q[:, hs]),
            "k": np.ascontiguousarray(k[:, hs]),
            "v": np.ascontiguousarray(v[:, hs]),
            "omask": np.ascontiguousarray(omask[hs]),
        })
    res = run_bass_kernel_spmd(nc, in_maps, core_ids=list(range(N_CORES)),
                               trace=_trace)
    out = np.concatenate([res.results[c]["out"] for c in range(N_CORES)],
                         axis=1)
    if _trace:
        kernel.last_results = res
    return out



# revision 25
# speedup vs baseline: 1.0752x; 1.0752x over previous
"""ParallelRetention Trainium2 Bass kernel (v3).

Problem (per [b,h] slice, B=2 H=16 S=2048 D=64):
    decay  = omask / sqrt(rowsum(omask))          (per-row rsqrt scale)
    ret    = (q @ k^T) * decay
    denom  = clip(|rowsum(ret)|, 1, inf)
    out    = (ret / denom) @ v
Restructured:
    augT   = [v | 1]^T @ (scores * omask)^T       # [65, S] per (b,h)
    msum   = rowsum(omask) = [0..0|1]^T @ omask^T  # extra PE stream
    out[q] = augT[0:64, q] / max(|augT[64, q]|, sqrt(msum[q]))

Structure (v6, ~275us; v5 was ~287us, original baseline 372us):
  - Scores computed transposed ([k, q]) with k-pairs row-packed so the two
    K=64 matmuls of a pair run in disjoint PE row groups. qT/kT are fp16
    (ACT casts on the PSUM->SBUF copy): the pair's two moving streams
    share the 4B/lane/cycle SBUF->PE bus, so 2-byte operands let the pair
    stream in ~316ns instead of fp32's 428ns. fp16 (not bf16: bf16 q/k
    fails the 2e-2 gate at 0.032; fp16 lands at 0.0043).
  - omaskT built by PE transposes + ACT PSUM->SBUF copies, software-
    pipelined into the previous chunk's matmul stream (build for ch+1
    interleaves with stream of ch); omask DMA prefetches a chunk ahead.
  - score*omask multiplies on DVE directly from PSUM (f32r out); the
    second (aug) matmul consumes them with [v|1] stationary; aug matmuls
    are deferred AUG_DEFER jj-slots so the PE never waits on the DVE.
  - msum comes from a second matmul stream over the omaskT tiles with an
    M=66 [0...0|1] stationary: cols 0-64 add zeros onto the aug rows and
    col 65 accumulates rowsum(omask) into partition 65 of b0's aug bank -
    no ACT/DVE row-sum pass at all. The postproc transpose carries msum
    out alongside augT (its matmuls run start=False onto an explicitly
    zeroed row, and begin only after both aug-group starts).
  - Each chunk's postprocess is deferred into the next chunk's stream
    (issued at jj=0, before that chunk's first deferred aug flush, which
    keeps aug-bank reuse ordering correct); qT/kT/va are triple-buffered
    so the next head's prep overlaps the current head's tail.
  - GPSIMD offload and DVE-accumulate msum were measured and are net
    losses (PSUM-copy tax / 1x CACHE_REDUCE); knobs remain for reference.

Sharding: 16 heads / 8 cores = 2 heads per core, both batches on the same
core (omask is per-head, halving omask traffic per core). SPMD: one NEFF,
per-core input slices.
"""

import os

import numpy as np

B = 2
H = 16
S = 2048
D = 64
N_CORES = 8
HC = H // N_CORES
P = 128
QT = S // P
KT = S // P
QC = 512
NCH = S // QC
TPC = QC // P

GP_QUADS = int(os.environ.get("KRN_GP_QUADS", "0"))   # 0..8 gpsimd quads/chunk
GP_DEFER = int(os.environ.get("KRN_GP_DEFER", "2"))   # jj slots to defer quad
AUG_DEFER = int(os.environ.get("KRN_AUG_DEFER", "3"))  # jj slots to defer the
                                                       # DVE-path aug MMs
SG8 = os.environ.get("KRN_SG8", "0") == "1"           # 2 k-blocks per stage
PIPE = os.environ.get("KRN_PIPE", "1") == "1"         # pipelined build
MSUM_MM = os.environ.get("KRN_MSUM_MM", "1") == "1"   # msum via PE stream
MSUM_DVE = int(os.environ.get("KRN_MSUM_DVE", "1"))   # msum tiles/chunk on
                                                      # DVE when MSUM_MM=0
N_WARMUP = int(os.environ.get("KRN_WARMUP", "12"))
FILLERS = int(os.environ.get("KRN_FILLERS", "0"))

_NC_CACHE = {}


def _build_nc():
    import concourse.mybir as mybir
    import concourse.tile as tile
    from concourse import bacc
    from concourse.masks import make_identity

    F32R = mybir.dt.float32r
    F32 = mybir.dt.float32
    BF16 = mybir.dt.bfloat16
    FP16 = mybir.dt.float16
    MULT = mybir.AluOpType.mult
    ADD = mybir.AluOpType.add

    KPG = 2 if SG8 else 1          # k-blocks per transpose stage group
    NGRP = KT // KPG               # stage groups per chunk (8 or 16)
    PGSZ = 8 if SG8 else 4         # transposes per prep stage group
    MCOL = 64                      # PE array col position of the msum
                                   # stationary; msum row = partition 65

    # build-step placement across the 8 jj slots of the previous chunk's
    # stream: group counts per jj (sums to NGRP)
    if SG8:
        GRP_AT = [0, 0, 2, 1, 1, 1, 1, 2]
    else:
        GRP_AT = [0, 0, 2, 3, 3, 3, 3, 2]
    MS_AT = [0, 0, 3, 3, 3, 3, 2, 2]     # msum matmuls per jj (sums to 16)
    MSUM_AT = {3: 0, 5: 1, 6: 2, 7: 3}   # jj -> onat tile (MSUM_MM=0 path)

    nc = bacc.Bacc("TRN2", target_bir_lowering=False, debug=False,
                   num_devices=N_CORES)

    q_d = nc.dram_tensor("q", [B, HC, S, D], F32, kind="ExternalInput")
    k_d = nc.dram_tensor("k", [B, HC, S, D], F32, kind="ExternalInput")
    v_d = nc.dram_tensor("v", [B, HC, S, D], F32, kind="ExternalInput")
    om_d = nc.dram_tensor("omask", [HC, S, S], F32, kind="ExternalInput")
    out_d = nc.dram_tensor("out", [B, HC, S, D], F32, kind="ExternalOutput")

    with tile.TileContext(nc) as tc:
        with (
            tc.tile_pool(name="const", bufs=1) as const_pool,
            tc.tile_pool(name="onat", bufs=4) as onat_pool,
            tc.tile_pool(name="omt", bufs=3) as omt_pool,
            tc.tile_pool(name="qkv", bufs=2) as qkv_pool,
            tc.tile_pool(name="work", bufs=min(4 + 2 * AUG_DEFER, 8)) as work_pool,
            tc.tile_pool(name="quad", bufs=2) as quad_pool,
            tc.tile_pool(name="small", bufs=4) as small_pool,
            tc.tile_pool(name="outp", bufs=2) as out_pool,
            tc.tile_pool(name="ps_sc", bufs=2, space="PSUM") as ps_sc,
            tc.tile_pool(name="ps_stage", bufs=(1 if SG8 else 2),
                         space="PSUM") as ps_stage,
            tc.tile_pool(name="ps_aug", bufs=2, space="PSUM") as ps_aug,
        ):
            ident_f = const_pool.tile([P, P], F32, tag="ident_f")
            make_identity(nc, ident_f)

            # [0 ... 0 | 1] stationary (M=66) for the msum matmul stream in
            # plain 128x128 mode: cols 0-64 add +0 onto the aug rows, col
            # 65 accumulates rowsum(omask) into partition 65. f32r so the
            # stream runs in single-pass full-rate fp32 mode.
            z66_f = const_pool.tile([P, D + 2], F32, tag="z66_f")
            nc.vector.memset(z66_f, 0.0)
            nc.vector.memset(z66_f[:, D + 1:D + 2], 1.0)
            z66 = const_pool.tile([P, D + 2], F32R, tag="z66")
            nc.vector.tensor_copy(z66, z66_f)
            tinyb = const_pool.tile([P, 1], F32, tag="tinyb")
            nc.vector.memset(tinyb, 1e-30)

            # dummy outputs for the MSUM_MM=0 accumulate path (never read)
            mdum_dve = None
            mdum_act = None
            if not MSUM_MM:
                F8 = mybir.dt.float8e4
                if MSUM_DVE > 0:
                    mdum_dve = const_pool.tile([P, S], F8, tag="mdum_dve")
                if MSUM_DVE < TPC:
                    mdum_act = const_pool.tile([P, S], F8, tag="mdum_act")

            # PE warmup: back-to-back matmuls so the HAM clock gate lifts
            # (1.2 -> 2.4 GHz) before the real stream; overlaps first DMAs.
            warm_w = const_pool.tile([P, P], F32R, tag="warm_w")
            nc.vector.tensor_copy(warm_w, ident_f)
            warm_xf = const_pool.tile([P, QC], F32, tag="warm_xf")
            nc.vector.memset(warm_xf, 1.0)
            warm_x = const_pool.tile([P, QC], F32R, tag="warm_x")
            nc.vector.tensor_copy(warm_x, warm_xf)
            if N_WARMUP:
                warm_ps = ps_aug.tile([P, QC], F32, tag="aug")
                for _ in range(N_WARMUP):
                    nc.tensor.matmul(warm_ps, warm_w, warm_x,
                                     start=True, stop=True)
                warm_sink = small_pool.tile([P, 1], F32, tag="warm_sink")
                nc.vector.tensor_copy(warm_sink, warm_ps[:, 0:1])

            def prep_inputs(b, h):
                # q tiles with the d-column block duplicated ([p, t, 128] =
                # [q | q]) and k tiles packed pairwise; a [128,128] PE
                # transpose of each yields qT duplicated into both partition
                # halves and kT pairs split 0-63/64-127 so the two K=64
                # score matmuls of a pair run in disjoint PE row-groups.
                qsrc = q_d[b, h].rearrange("(t p) d -> p t d", p=P)
                qn2 = qkv_pool.tile([P, QT, P], F32, tag="qn")
                nc.sync.dma_start(out=qn2[:, :, 0:D], in_=qsrc)
                nc.sync.dma_start(out=qn2[:, :, D:2 * D], in_=qsrc)
                kn2 = qkv_pool.tile([P, KT // 2, 2, D], F32, tag="kn")
                nc.sync.dma_start(
                    out=kn2,
                    in_=k_d[b, h].rearrange(
                        "(jj two p) d -> p jj two d", p=P, two=2))
                vn = qkv_pool.tile([P, KT, D], F32, tag="vn")
                nc.sync.dma_start(
                    out=vn,
                    in_=v_d[b, h].rearrange("(t p) d -> p t d", p=P))
                va = qkv_pool.tile([P, KT, D + 1], F32R, tag="va", bufs=3)
                nc.vector.tensor_copy(va[:, :, 0:D], vn)
                onesf = small_pool.tile([P, KT], F32, tag="onesf")
                nc.vector.memset(onesf, 1.0)
                nc.vector.tensor_copy(
                    va[:, :, D:D + 1].rearrange("p t o -> p (t o)"), onesf)

                # qT/kT in bf16: halves the score matmuls' moving-stream
                # bytes so the two row-group-packed K=64 matmuls of a pair
                # can stream concurrently
                qT = qkv_pool.tile([P, S], FP16, tag="qT", bufs=3)
                for g in range(QT // PGSZ):
                    stg = ps_stage.tile([P, PGSZ, P], F32, tag="stage")
                    for i in range(PGSZ):
                        nc.tensor.transpose(
                            stg[:, i, :], qn2[:, g * PGSZ + i, :], ident_f)
                    nc.scalar.copy(
                        out=qT[:, g * PGSZ * P:(g + 1) * PGSZ * P]
                            .rearrange("d (i c) -> d i c", c=P),
                        in_=stg)
                kT = qkv_pool.tile([P, KT // 2, P], FP16, tag="kT", bufs=3)
                kg = min(PGSZ, KT // 2)
                for g in range((KT // 2) // kg):
                    stg = ps_stage.tile([P, kg, P], F32, tag="stage")
                    for i in range(kg):
                        nc.tensor.transpose(
                            stg[:, i, :],
                            kn2[:, g * kg + i, :, :]
                                .rearrange("p two d -> p (two d)"),
                            ident_f)
                    nc.scalar.copy(
                        out=kT[:, g * kg:(g + 1) * kg, :], in_=stg)
                return qT, kT, va

            # GPSIMD quad assignment: (b, quad) pairs, spread across jj & b
            gps_set = set(
                [(0, 1), (1, 2), (1, 0), (0, 2), (0, 0), (1, 1), (0, 3),
                 (1, 3)][:GP_QUADS])

            # postproc closures deferred into the next chunk's stream so
            # their ACT/DVE/PE chains overlap instead of serializing at
            # chunk boundaries
            pending_post = []

            def flush_post():
                while pending_post:
                    pending_post.pop(0)()

            for h in range(HC):
                prepped = [prep_inputs(b, h) for b in range(B)]
                msum = None
                if not MSUM_MM:
                    msum = small_pool.tile([P, QT], F32, tag="msum")

                def msum_op(onats, ch, t):
                    qt = ch * TPC + t
                    if t < MSUM_DVE:
                        nc.vector.tensor_scalar(
                            mdum_dve, onats[t], 1.0, 0.0, MULT, ADD,
                            accum_out=msum[:, qt:qt + 1])
                    else:
                        nc.scalar.activation(
                            mdum_act, onats[t],
                            mybir.ActivationFunctionType.Copy,
                            accum_out=msum[:, qt:qt + 1])

                def issue_onat_dma(ch):
                    onats = []
                    for t in range(TPC):
                        onat = onat_pool.tile([P, S], F32, tag="onat")
                        r0 = ch * QC + t * P
                        nc.sync.dma_start(
                            out=onat, in_=om_d[h, r0:r0 + P, :])
                        onats.append(onat)
                    return onats

                def alloc_omts():
                    # t-major layout [P, t, j8, c]: each build step (one
                    # onat row-tile, 4 k-blocks) writes a CONTIGUOUS
                    # [P, 4, 128] slab, and a step depends on only ONE
                    # omask DMA tile, so the build starts as soon as the
                    # first tile lands instead of waiting for all four.
                    return [omt_pool.tile([P, TPC, KT // 2, P], F32R,
                                          tag="omt", name=f"omt{i}")
                            for i in range(2)]

                def make_group_steps(onats, omts):
                    """16 closures, one per (row-tile t, k-quad jb)."""
                    def make_step(t, jb):
                        def step():
                            j0 = jb * 4
                            stg = ps_stage.tile([P, 4, P], F32, tag="stage")
                            for i in range(4):
                                j = j0 + i
                                nc.tensor.transpose(
                                    stg[:, i, :],
                                    onats[t][:, j * P:(j + 1) * P],
                                    ident_f)
                            half = j0 // (KT // 2)
                            j8 = j0 % (KT // 2)
                            # the stage->omt copies are the build's
                            # throughput limit (ACT ~730ns vs 440ns of
                            # transposes per stage); send every 4th to
                            # the DVE to unclog the stage-buffer pipeline
                            dst = omts[half][:, t, j8:j8 + 4, :]
                            if (t * 4 + jb) % 4 == 3:
                                nc.vector.tensor_copy(dst, stg)
                            else:
                                nc.scalar.copy(out=dst, in_=stg)
                        return step
                    return [make_step(t, jb)
                            for t in range(TPC) for jb in range(4)]

                def full_build(ch):
                    onats = issue_onat_dma(ch)
                    omts = alloc_omts()
                    for step in make_group_steps(onats, omts):
                        step()
                    if not MSUM_MM:
                        for t in range(TPC):
                            msum_op(onats, ch, t)
                    return omts

                # chunk 0 of each head: built as its own phase (overlaps
                # warmup / previous head's stream tail via Tile scheduling)
                omts = full_build(0)

                for ch in range(NCH):
                    gsteps, omts_n, onats_n = [], None, None
                    if ch + 1 < NCH:
                        onats_n = issue_onat_dma(ch + 1)
                        omts_n = alloc_omts()
                        gsteps = make_group_steps(onats_n, omts_n)
                        if not PIPE:
                            for step in gsteps:
                                step()
                            gsteps = []

                    def omt_j(j):
                        # [P, t, c] view of k-block j (strided over t)
                        half = j // (KT // 2)
                        j8 = j % (KT // 2)
                        return omts[half][:, :, j8, :]

                    def omt_at(jj, n=2):
                        # [P, j, t, c] view matching the score pair layout
                        half = (jj * 2) // (KT // 2)
                        j8 = (jj * 2) % (KT // 2)
                        return omts[half][:, :, j8:j8 + n, :].rearrange(
                            "p t j c -> p j t c")

                    aug_ps = {}
                    pend = {}
                    started = {}
                    issued = {b: 0 for b in range(B)}
                    deferred = []
                    for b in range(B):
                        aug_b = ps_aug.tile([P, QC], F32, tag="aug",
                                            name=f"aug{b}")
                        aug_ps[b] = aug_b
                        pend[b] = None
                        started[b] = False

                    def mm2(b, j, src_ap):
                        va = prepped[b][2]
                        issued[b] += 1
                        nc.tensor.matmul(
                            aug_ps[b][0:D + 1, :], va[:, j, :], src_ap,
                            start=not started[b],
                            stop=(issued[b] == KT),
                            skip_group_check=True)
                        started[b] = True

                    def flush_deferred(jj_now):
                        for item in list(deferred):
                            fjj, b, mms = item
                            if jj_now < fjj:
                                continue
                            for j, ap in mms:
                                mm2(b, j, ap)
                            deferred.remove(item)

                    gi = 0
                    ms_done = 0
                    for jj in range(KT // 2):
                        for b in range(B):
                            qT, kT, va = prepped[b]
                            sc = ps_sc.tile([P, 2, QC], F32, tag="scores")
                            for j2 in range(2):
                                base = j2 * D
                                nc.tensor.matmul(
                                    sc[:, j2, :], kT[base:base + D, jj, :],
                                    qT[base:base + D,
                                       ch * QC:(ch + 1) * QC],
                                    start=True, stop=True)
                            if (b, jj // 2) in gps_set:
                                if pend[b] is None:
                                    scq = quad_pool.tile([P, 4, QC], F32,
                                                         tag="scq")
                                    retq = quad_pool.tile([P, 4, QC], F32R,
                                                          tag="retq")
                                    pend[b] = (scq, retq, jj)
                                scq, retq, jj0 = pend[b]
                                off = (jj - jj0) * 2
                                nc.scalar.copy(
                                    out=scq[:, off:off + 2, :], in_=sc)
                                if off == 2:
                                    nc.gpsimd.tensor_mul(
                                        retq.rearrange(
                                            "p j (t c) -> p j t c", c=P),
                                        scq.rearrange(
                                            "p j (t c) -> p j t c", c=P),
                                        omt_at(jj0, 4))
                                    deferred.append((
                                        jj + GP_DEFER, b,
                                        [(jj0 * 2 + jq, retq[:, jq, :])
                                         for jq in range(4)]))
                                    pend[b] = None
                            else:
                                ret = work_pool.tile([P, 2, QC], F32R,
                                                     tag="ret")
                                nc.vector.tensor_mul(
                                    ret.rearrange("p j (t c) -> p j t c",
                                                  c=P),
                                    sc.rearrange("p j (t c) -> p j t c",
                                                 c=P),
                                    omt_at(jj))
                                mms = [(jj * 2 + j2, ret[:, j2, :])
                                       for j2 in range(2)]
                                if AUG_DEFER:
                                    deferred.append(
                                        (jj + AUG_DEFER, b, mms))
                                else:
                                    for j, ap in mms:
                                        mm2(b, j, ap)
                            flush_deferred(jj)
                        if jj == 0:
                            # previous chunk's postproc drops in here so it
                            # overlaps this stream. Must precede this
                            # chunk's first aug flush (jj>=1) so the aug
                            # PSUM buffer reuse ordering stays correct.
                            flush_post()
                            if MSUM_MM:
                                # zero the msum accumulator row (its
                                # matmuls all run start=False). Partition
                                # base must be 32-aligned: clear 64-65; row
                                # 64 is re-initialized by the aug group's
                                # start.
                                nc.vector.memset(
                                    aug_ps[0][MCOL:MCOL + 2, :], 0.0)
                        if MSUM_MM:
                            # msum stream: M=66 [0...0|1] stationary
                            # accumulating rowsum(omask) into partition 65
                            # of b0's aug bank (cols 0-64 add zeros). All
                            # start=False onto the explicitly-zeroed
                            # partition; begins at jj=2, after both
                            # aug-group starts have executed.
                            n_ms = MS_AT[jj]
                            for j in range(ms_done, ms_done + n_ms):
                                nc.tensor.matmul(
                                    aug_ps[0][0:D + 2, :], z66,
                                    omt_j(j),
                                    start=False, stop=(j == KT - 1),
                                    skip_group_check=True)
                            ms_done += n_ms
                        if PIPE and gsteps:
                            for _ in range(GRP_AT[jj]):
                                if gi < len(gsteps):
                                    gsteps[gi]()
                                    gi += 1
                        if (not MSUM_MM and onats_n is not None
                                and jj in MSUM_AT):
                            msum_op(onats_n, ch + 1, MSUM_AT[jj])
                        if FILLERS:
                            fill = ps_stage.tile([P, D], F32, tag="stage")
                            for _ in range(FILLERS):
                                nc.tensor.matmul(
                                    fill, warm_w, warm_x[:, 0:D],
                                    start=True, stop=True)
                    flush_deferred(10 ** 9)

                    def make_post(h, ch, aug_ps, msum):
                        def post():
                            msq = small_pool.tile([P, TPC], F32, tag="msq")
                            for b in range(B):
                                # postproc: [65, QC] (+ msum row) -> scaled
                                # [q, d] output
                                hi = (MCOL + 2 if (b == 0 and MSUM_MM)
                                      else D + 1)
                                augs = out_pool.tile([P, QC], F32,
                                                     tag="augs")
                                nc.scalar.copy(out=augs[0:hi, :],
                                               in_=aug_ps[b][0:hi, :])
                                autp = ps_stage.tile([P, TPC, MCOL + 2],
                                                     F32, tag="stage")
                                for t in range(TPC):
                                    nc.tensor.transpose(
                                        autp[:, t, 0:hi],
                                        augs[0:hi, t * P:(t + 1) * P],
                                        ident_f[0:hi, 0:hi])
                                if b == 0 and MSUM_MM:
                                    # msq = sqrt(msum + tiny)
                                    nc.scalar.activation(
                                        msq,
                                        autp[:, :, MCOL + 1:MCOL + 2]
                                        .rearrange("p t o -> p (t o)"),
                                        mybir.ActivationFunctionType.Sqrt,
                                        bias=tinyb[:, 0:1])
                                elif b == 0:
                                    nc.scalar.activation(
                                        msq,
                                        msum[:, ch * TPC:(ch + 1) * TPC],
                                        mybir.ActivationFunctionType.Sqrt,
                                        bias=tinyb[:, 0:1])
                                scal = small_pool.tile([P, TPC], F32,
                                                       tag="scal",
                                                       name=f"scal{b}")
                                nc.scalar.activation(
                                    scal,
                                    autp[:, :, D:D + 1].rearrange(
                                        "p t o -> p (t o)"),
                                    mybir.ActivationFunctionType.Abs)
                                nc.vector.tensor_max(scal, scal, msq)
                                nc.vector.reciprocal(scal, scal)
                                ob = out_pool.tile([P, TPC, D], F32,
                                                   tag="ob")
                                for t in range(TPC):
                                    # per-partition scale: alternate
                                    # ACT/DVE (8 serialized ACT ops were
                                    # the postproc tail)
                                    if t % 2:
                                        nc.vector.tensor_scalar(
                                            ob[:, t, :], autp[:, t, 0:D],
                                            scal[:, t:t + 1], 0.0,
                                            MULT, ADD)
                                    else:
                                        nc.scalar.activation(
                                            ob[:, t, :], autp[:, t, 0:D],
                                            mybir.ActivationFunctionType
                                            .Copy,
                                            scale=scal[:, t:t + 1])
                                nc.sync.dma_start(
                                    out=out_d[b, h,
                                              ch * QC:(ch + 1) * QC, :]
                                    .rearrange("(t p) d -> p t d", p=P),
                                    in_=ob)
                        return post

                    pending_post.append(make_post(h, ch, aug_ps, msum))

                    omts = omts_n
            flush_post()

    nc.compile()
    return nc


def _get_nc():
    if "nc" not in _NC_CACHE:
        _NC_CACHE["nc"] = _build_nc()
    return _NC_CACHE["nc"]


def kernel(q, k, v, omask, _trace=False):
    from concourse.bass_utils import run_bass_kernel_spmd

    nc = _get_nc()
    in_maps = []
    for c in range(N_CORES):
        hs = slice(c * HC, (c + 1) * HC)
        in_maps.append({
            "q": np.ascontiguousarray(q[:, hs]),
            "k": np.ascontiguousarray(k[:, hs]),
            "v": np.ascontiguousarray(v[:, hs]),
            "omask": np.ascontiguousarray(omask[hs]),
        })
    res = run_bass_kernel_spmd(nc, in_maps, core_ids=list(range(N_CORES)),
                               trace=_trace)
    out = np.concatenate([res.results[c]["out"] for c in range(N_CORES)],
                         axis=1)
    if _trace:
        kernel.last_results = res
    return out

